# revision 1
# baseline (speedup 1.0000x reference)
"""Trainium2 Bass kernel for nn_BertSelfAttention_79577154060613.

Block-sparse BERT self-attention (block-diagonal over 10 candidate blocks of
64 tokens + dense global columns for 128 term tokens), data-parallel over
batch across 8 NeuronCores (2 batches per core).

Key algorithmic trick: the reference multiplies scores by the mask (masked
entries become exactly 0, not -inf), so softmax gives each masked key weight
exp(0)=1. For a query in block c:
    ctx = (sum_{k in block c | terms} e^{s_k} v_k + sum_{c' != c} Vsum_c') / Z
    Z   = sum_{k in block c | terms} e^{s_k} + 9*64
where Vsum_c' are per-head, per-block sums of candidate value rows. This
turns 768-wide attention into 192-wide attention plus one small K=10 matmul
(lhsT = 1 - one_hot(c)) per query tile.

All tensor-engine inputs are bf16 (fp32 matmuls stream at 1/4 rate on TRN2);
accumulation stays fp32 in PSUM and the softmax divide runs in fp32, so the
end-to-end error stays at the bf16-rounding level (~2e-3 relative).

Layouts (no on-chip transposes anywhere):
  - X^T [h, t]: host pre-transposes and pre-casts hidden_states.
  - Q^T, K^T [o, t] = matmul(lhsT=W^T tile, rhs=X^T); head h lives at
    partitions (h%2)*64 of tile h//2. Biases are added by the PSUM->SBUF
    copy (per-partition tensor_scalar add).
  - V [t, o] = matmul(lhsT=X^T tile, rhs=W^T), stored per head with a ones
    column ([t, 12*(64+1)] bf16) so every PV matmul also accumulates the
    softmax denominator into a 65th PSUM column. V's bias (free-dim) is
    added via a materialized [128, H] bias tile on the copy.
  - scores^T [k, q] = matmul(lhsT=K^T head, rhs=Q^T head); exp on ACT
    (scale=1/8) casting to bf16; the exp'ed scores are the *stationary*
    operand of PV, giving ctx in natural [q, dh] layout, so the divided
    output DMAs straight out.

PSUM discipline: start=True lazily zeroes the whole 2KB bank for the written
partitions, so each bank hosts exactly one accumulation group per partition
half, opened by the correction matmuls (which cover all 4 heads' columns).
"""

import numpy as np
import ml_dtypes

import concourse.bass as bass
import concourse.mybir as mybir
import concourse.tile as tile
from concourse import bacc
from concourse.bass_utils import run_bass_kernel_spmd

# Problem dims (hardcoded per contract)
B, CDD, L, T, H, NH = 16, 10, 64, 128, 768, 12
DH = H // NH  # 64
S = CDD * L + T  # 768
NQ = CDD * L  # 640
P = 128
NCORES = 8
BL = B // NCORES  # 2 batches per core
KT = H // P  # 6 contraction tiles
FP32 = mybir.dt.float32
BF16 = mybir.dt.bfloat16
AF = mybir.ActivationFunctionType
ALU = mybir.AluOpType
HGS = 4  # heads per attention group
NHG = NH // HGS  # 3 groups
VW = DH + 1  # value width per head incl. ones column (65)


def _build_program():
    nc = bacc.Bacc(
        "TRN2", target_bir_lowering=False, debug=False, num_devices=NCORES
    )
    x = nc.dram_tensor("x", [BL, H, S], BF16, kind="ExternalInput").ap()
    wqt = nc.dram_tensor("wqt", [H, H], BF16, kind="ExternalInput").ap()
    wkt = nc.dram_tensor("wkt", [H, H], BF16, kind="ExternalInput").ap()
    wvt = nc.dram_tensor("wvt", [H, H], BF16, kind="ExternalInput").ap()
    bq = nc.dram_tensor("bq", [H], FP32, kind="ExternalInput").ap()
    bk = nc.dram_tensor("bk", [H], FP32, kind="ExternalInput").ap()
    bv16 = nc.dram_tensor("bv16", [H], BF16, kind="ExternalInput").ap()
    out = nc.dram_tensor("out", [BL, S, H], FP32, kind="ExternalOutput").ap()

    with tile.TileContext(nc) as tc:
        _emit(tc, nc, x, wqt, wkt, wvt, bq, bk, bv16, out)
    nc.compile()
    return nc


def _emit(tc, nc, x, wqt, wkt, wvt, bq, bk, bv16, out):
    from contextlib import ExitStack

    ctx = ExitStack()
    with ctx:
        cpool = ctx.enter_context(tc.tile_pool(name="consts", bufs=1))
        wpool = ctx.enter_context(tc.tile_pool(name="weights", bufs=1))
        xtp = ctx.enter_context(tc.tile_pool(name="xt", bufs=2))
        qkv = ctx.enter_context(tc.tile_pool(name="qkv", bufs=2))
        sep = ctx.enter_context(tc.tile_pool(name="se", bufs=3))
        osp = ctx.enter_context(tc.tile_pool(name="osb", bufs=1))
        smp = ctx.enter_context(tc.tile_pool(name="small", bufs=2))
        psp = ctx.enter_context(tc.tile_pool(name="psum", bufs=1, space="PSUM"))

        # ---- constants ----
        onesrow = cpool.tile([1, P], BF16)  # 1.0 row (rank-1 lhsT)
        nc.gpsimd.memset(onesrow[:], 1.0)
        zrow = cpool.tile([1, 1], BF16)  # 0.0 (group-closer rank-1 rhs)
        nc.gpsimd.memset(zrow[:], 0.0)
        # notselC[p, c*64+j] = 0 if p == c else 1  (p in 0..9)
        notselC = cpool.tile([CDD, NQ], BF16)
        nc.gpsimd.memset(notselC[:], 1.0)
        nc.gpsimd.affine_select(
            out=notselC.rearrange("p (c j) -> p c j", j=L),
            in_=notselC.rearrange("p (c j) -> p c j", j=L),
            compare_op=ALU.not_equal,
            fill=0.0,
            base=0,
            pattern=[[-1, CDD], [0, L]],
            channel_multiplier=1,
        )
        # block-membership indicator for Vsums: G[p, j] = 1 iff j-10 == p//64
        G = cpool.tile([P, 20], BF16)
        nc.gpsimd.memset(G[:], 0.0)
        nc.gpsimd.memset(G[0:64, 10:11], 1.0)
        nc.gpsimd.memset(G[64:128, 11:12], 1.0)

        # ---- weights & biases (shared by both batches) ----
        # xt(b=0) + wq chunks are interleaved so the first projection's
        # K-accumulation can start as soon as chunk 0 lands; wk/wv follow.
        w_sb = {}
        w_aps = {"q": wqt, "k": wkt, "v": wvt}
        for name in ("q", "k", "v"):
            w_sb[name] = wpool.tile(
                [P, KT, H], BF16, tag=f"w{name}", name=f"w{name}"
            )
        bvb = cpool.tile([P, H], FP32)  # built right before V projection
        b_col = {}
        bv_row = cpool.tile([1, H], BF16)
        xt0 = []
        for kt in range(KT):
            t = xtp.tile([P, S], BF16, tag=f"xt{kt}", name=f"xt{kt}")
            nc.sync.dma_start(out=t[:], in_=x[0][kt * P : (kt + 1) * P, :])
            nc.sync.dma_start(
                out=w_sb["q"][:, kt, :],
                in_=w_aps["q"].rearrange("(kt p) o -> p kt o", p=P)[:, kt, :],
            )
            xt0.append(t)
            if kt == 0:
                # tiny bias DMAs (needed by the first projection copies)
                for name, bap in (("q", bq), ("k", bk)):
                    bc = cpool.tile([P, KT], FP32, tag=f"bc{name}", name=f"bcol{name}")
                    nc.sync.dma_start(
                        out=bc[:], in_=bap.rearrange("(t p) -> p t", p=P)
                    )
                    b_col[name] = bc
                nc.sync.dma_start(out=bv_row[:], in_=bv16[None, :])
        for name in ("k", "v"):
            wr = w_aps[name].rearrange("(kt p) o -> p kt o", p=P)
            for kt in range(KT):
                nc.sync.dma_start(out=w_sb[name][:, kt, :], in_=wr[:, kt, :])

        xt_next = xt0
        for b in range(BL):
            # ---- X^T (host pre-transposed, bf16; prefetched) ----
            xt = xt_next

            # ---- projections ----
            qt_sb = [qkv.tile([P, NQ], BF16, tag=f"qt{m}", name=f"qt{m}") for m in range(KT)]
            kt_sb = [qkv.tile([P, S], BF16, tag=f"kt{m}", name=f"kt{m}") for m in range(KT)]
            vext = [qkv.tile([P, NH * VW], BF16, tag=f"v{m}", name=f"v{m}") for m in range(KT)]
            vterm = qkv.tile([P, H], FP32, tag="vterm", name="vterm")

            # Q^T, K^T: out[o-tile, t-chunk]; bias added on the copy
            for name, dst, nlen_total in (("q", qt_sb, NQ), ("k", kt_sb, S)):
                for mt in range(KT):
                    ms = slice(mt * P, (mt + 1) * P)
                    n0 = 0
                    while n0 < nlen_total:
                        nlen = min(512, nlen_total - n0)
                        ps = psp.tile([P, 512], FP32, tag="psA", bufs=4, name="psA")
                        for kt in range(KT):
                            nc.tensor.matmul(
                                ps[:, :nlen],
                                lhsT=w_sb[name][:, kt, ms],
                                rhs=xt[kt][:, n0 : n0 + nlen],
                                start=(kt == 0),
                                stop=(kt == KT - 1),
                            )
                        if name == "q":
                            nc.scalar.activation(
                                dst[mt][:, n0 : n0 + nlen],
                                ps[:, :nlen],
                                AF.Identity,
                                bias=b_col[name][:, mt : mt + 1],
                            )
                        else:
                            nc.vector.tensor_scalar_add(
                                dst[mt][:, n0 : n0 + nlen],
                                ps[:, :nlen],
                                b_col[name][:, mt : mt + 1],
                            )
                        n0 += nlen

            if b == 0:
                # materialized V bias [128, H] fp32 (free-dim bias add on copy)
                for n0, nlen in ((0, 512), (512, 256)):
                    ps = psp.tile([P, 512], FP32, tag="psA", bufs=4, name="psA")
                    nc.tensor.matmul(
                        ps[:, :nlen],
                        lhsT=onesrow[:],
                        rhs=bv_row[0:1, n0 : n0 + nlen],
                        start=True,
                        stop=True,
                    )
                    nc.vector.tensor_copy(bvb[:, n0 : n0 + nlen], ps[:, :nlen])

            # V: out[t-tile, o-chunk] -> vext (bf16, 65-strided) + vterm fp32
            for mt in range(KT):
                ms = slice(mt * P, (mt + 1) * P)
                for n0, nlen in ((0, 512), (512, 256)):
                    ps = psp.tile([P, 512], FP32, tag="psA", bufs=4, name="psA")
                    for kt in range(KT):
                        nc.tensor.matmul(
                            ps[:, :nlen],
                            lhsT=xt[kt][:, ms],
                            rhs=w_sb["v"][:, kt, n0 : n0 + nlen],
                            start=(kt == 0),
                            stop=(kt == KT - 1),
                        )
                    nh0 = n0 // DH
                    nheads = nlen // DH
                    vv = vext[mt].rearrange("p (h c) -> p h c", c=VW)
                    nc.vector.tensor_tensor(
                        out=vv[:, nh0 : nh0 + nheads, 0:DH],
                        in0=ps[:, :nlen].rearrange("p (h c) -> p h c", c=DH),
                        in1=bvb[:, n0 : n0 + nlen].rearrange("p (h c) -> p h c", c=DH),
                        op=ALU.add,
                    )
                    if mt == KT - 1:
                        # fp32 copy of term-value rows for output passthrough
                        nc.scalar.activation(
                            vterm[:, n0 : n0 + nlen],
                            ps[:, :nlen],
                            AF.Copy,
                        )
                vv = vext[mt].rearrange("p (h c) -> p h c", c=VW)
                nc.gpsimd.memset(vv[:, :, DH : DH + 1], 1.0)
            # vterm still needs the bias
            nc.vector.tensor_add(vterm[:], vterm[:], bvb[:])
            # term rows pass through V (fp32, bias included) - DMA out early
            nc.sync.dma_start(out=out[b][NQ:S, :], in_=vterm[:])

            # ---- per-block value sums, stored with 65th col = 64.0 so the
            # notselC correction matmul also contributes 9*64=576 to Z ----
            vsumsE = smp.tile([CDD, NH * VW], BF16, tag="vsums", name="vsumsE")
            for n0 in (0, 384):
                ps = psp.tile([P, 512], FP32, tag="psA", bufs=4, name="psA")
                nh0 = n0 // DH
                for kt in range(5):
                    rhs = vext[kt].rearrange("p (h c) -> p h c", c=VW)[
                        :, nh0 : nh0 + 6, 0:DH
                    ]
                    nc.tensor.matmul(
                        ps[0:CDD, 0:384],
                        lhsT=G[:, 10 - 2 * kt : 20 - 2 * kt],
                        rhs=rhs,
                        start=(kt == 0),
                        stop=(kt == 4),
                    )
                vsv = vsumsE.rearrange("p (h c) -> p h c", c=VW)
                nc.vector.tensor_copy(
                    vsv[:, nh0 : nh0 + 6, 0:DH],
                    ps[0:CDD, 0:384].rearrange("p (h c) -> p h c", c=DH),
                )
            vsv = vsumsE.rearrange("p (h c) -> p h c", c=VW)
            nc.gpsimd.memset(vsv[:, :, DH : DH + 1], float(L))

            # prefetch next batch's X^T while attention runs (SWDGE path)
            if b + 1 < BL:
                xt_next = []
                for kt in range(KT):
                    t = xtp.tile([P, S], BF16, tag=f"xt{kt}", name=f"xt{kt}")
                    nc.sync.dma_start(
                        out=t[:], in_=x[b + 1][kt * P : (kt + 1) * P, :]
                    )
                    xt_next.append(t)

            # ---- attention ----
            def emit_scores(hg):
                se_t = [sep.tile([P, NQ], BF16, tag=f"set{i}", name=f"set{i}") for i in range(HGS)]
                se_b = [sep.tile([P, 5 * L], BF16, tag=f"seb{i}", name=f"seb{i}") for i in range(HGS)]
                for hl in range(HGS):
                    hh = hg * HGS + hl
                    pt, r0 = hh // 2, (hh % 2) * 64
                    QTh = qt_sb[pt][r0 : r0 + 64, :]
                    KTh = kt_sb[pt][r0 : r0 + 64, :]
                    # term scores^T [128 terms, 640 q]
                    for n0 in (0, 320):
                        ps = psp.tile([P, 512], FP32, tag="psA", bufs=4, name="psA")
                        nc.tensor.matmul(
                            ps[:, 0:320],
                            lhsT=KTh[:, NQ:S],
                            rhs=QTh[:, n0 : n0 + 320],
                            start=True,
                            stop=True,
                        )
                        nc.scalar.activation(
                            se_t[hl][:, n0 : n0 + 320],
                            ps[:, 0:320],
                            AF.Exp,
                            scale=0.125,
                        )
                    # block scores^T: all 10 blocks in one psum bank
                    ps = psp.tile([P, 5 * L], FP32, tag="psB", bufs=1, name="psB", padded_shape=[P, 512])
                    for j in range(5):
                        for half in (0, 1):
                            c = 2 * j + half
                            cs = slice(c * L, (c + 1) * L)
                            nc.tensor.matmul(
                                ps[half * 64 : half * 64 + 64, j * L : (j + 1) * L],
                                lhsT=KTh[:, cs],
                                rhs=QTh[:, cs],
                                start=True,
                                stop=True,
                            )
                    nc.scalar.activation(
                        se_b[hl][:],
                        ps[:],
                        AF.Exp,
                        scale=0.125,
                    )
                return se_t, se_b

            def emit_pv(hg, se_t, se_b):
                for j in range(5):
                    psc = psp.tile([P, HGS * VW], FP32, tag="psC", bufs=3, name="psC", padded_shape=[P, 512])
                    hgs_v = slice(hg * HGS * VW, (hg + 1) * HGS * VW)
                    # head 0's full-height terms matmul opens the bank's one
                    # accumulation group; everything else accumulates.
                    for hl in range(HGS):
                        hh = hg * HGS + hl
                        vs = slice(hh * VW, (hh + 1) * VW)
                        nc.tensor.matmul(
                            psc[:, hl * VW : (hl + 1) * VW],
                            lhsT=se_t[hl][:, j * P : (j + 1) * P],
                            rhs=vext[5][:, vs],
                            start=(hl == 0),
                            stop=False,
                        )
                    for half in (0, 1):
                        c = 2 * j + half
                        hs = slice(half * 64, half * 64 + 64)
                        nc.tensor.matmul(
                            psc[hs, :],
                            lhsT=notselC[:, c * L : (c + 1) * L],
                            rhs=vsumsE[:, hgs_v],
                            start=False,
                            stop=False,
                        )
                    for hl in range(HGS):
                        hh = hg * HGS + hl
                        c0 = hl * VW
                        vs = slice(hh * VW, (hh + 1) * VW)
                        for half in (0, 1):
                            hs = slice(half * 64, half * 64 + 64)
                            nc.tensor.matmul(
                                psc[hs, c0 : c0 + VW],
                                lhsT=se_b[hl][hs, j * L : (j + 1) * L],
                                rhs=vext[j][hs, vs],
                                start=False,
                                stop=False,
                            )
                    # full-height +0 rank-1 whose stop closes the bank's group
                    nc.tensor.matmul(
                        psc[:, DH : DH + 1],
                        lhsT=onesrow[:],
                        rhs=zrow[:],
                        start=False,
                        stop=True,
                    )
                    zr = smp.tile([P, HGS], FP32, tag="zr", bufs=4, name="zr")
                    pscv = psc.rearrange("p (h c) -> p h c", c=VW)
                    nc.vector.reciprocal(
                        zr[:].rearrange("p (h o) -> p h o", o=1),
                        pscv[:, :, DH : DH + 1],
                    )
                    ob = osp.tile([P, HGS * DH], FP32, tag=f"osb{j}", bufs=2, name=f"osb{j}")
                    in0 = pscv[:, :, 0:DH]
                    in1 = zr[:].rearrange("p (h o) -> p h o", o=1)
                    bin0, bin1 = bass.broadcast_tensor_aps(in0, in1)
                    nc.vector.tensor_tensor(
                        out=ob[:].rearrange("p (h c) -> p h c", c=DH),
                        in0=bin0,
                        in1=bin1,
                        op=ALU.mult,
                    )
                    nc.sync.dma_start(
                        out=out[b][j * P : (j + 1) * P, hg * HGS * DH : (hg + 1) * HGS * DH],
                        in_=ob[:],
                    )

            prev = None
            for hg in range(NHG):
                cur = emit_scores(hg)
                if prev is not None:
                    emit_pv(hg - 1, *prev)
                prev = cur
            emit_pv(NHG - 1, *prev)

_CACHE = {}


def _get_program():
    if "nc" not in _CACHE:
        _CACHE["nc"] = _build_program()
    return _CACHE["nc"]


def _make_in_maps(inputs):
    hs = np.asarray(inputs["hidden_states"], np.float32)
    hst = np.ascontiguousarray(hs.transpose(0, 2, 1)).astype(ml_dtypes.bfloat16)
    wq = np.asarray(inputs["Wq"], np.float32)
    wk = np.asarray(inputs["Wk"], np.float32)
    wv = np.asarray(inputs["Wv"], np.float32)
    in_common = {
        "wqt": np.ascontiguousarray(wq.T).astype(ml_dtypes.bfloat16),
        "wkt": np.ascontiguousarray(wk.T).astype(ml_dtypes.bfloat16),
        "wvt": np.ascontiguousarray(wv.T).astype(ml_dtypes.bfloat16),
        "bq": np.asarray(inputs["bq"], np.float32),
        "bk": np.asarray(inputs["bk"], np.float32),
        "bv16": np.asarray(inputs["bv"], np.float32).astype(ml_dtypes.bfloat16),
    }
    return [
        {"x": hst[i * BL : (i + 1) * BL], **in_common} for i in range(NCORES)
    ]


def kernel(**inputs) -> np.ndarray:
    in_maps = _make_in_maps(inputs)
    nc = _get_program()
    res = run_bass_kernel_spmd(nc, in_maps, list(range(NCORES)))
    return np.concatenate([res.results[i]["out"] for i in range(NCORES)], axis=0)



# revision 6
# speedup vs baseline: 1.2897x; 1.2897x over previous
"""Trainium2 Bass kernel for nn_BertSelfAttention_79577154060613.

Block-sparse BERT self-attention (block-diagonal over 10 candidate blocks of
64 tokens + dense global columns for 128 term tokens), data-parallel over
batch across 8 NeuronCores (2 batches per core).

Key algorithmic trick: the reference multiplies scores by the mask (masked
entries become exactly 0, not -inf), so softmax gives each masked key weight
exp(0)=1. For a query in block c:
    ctx = (sum_{k in block c | terms} e^{s_k} v_k + sum_{c' != c} Vsum_c') / Z
    Z   = sum_{k in block c | terms} e^{s_k} + 9*64
where Vsum_c' are per-head, per-block sums of candidate value rows. This
turns 768-wide attention into 192-wide attention plus one small K=10 matmul
(lhsT = 1 - one_hot(c)) per query tile.

Performance structure (vs the bf16 baseline):
  - All projection matmuls (Q, K, V-candidates, Vsum) run in fp8 e4m3 with
    MatmulPerfMode.DoubleRow: two 128-row contraction planes per
    instruction at 0.5 cycles/output-column (4x fewer PE cycles than bf16).
    X and W are cast to fp8 on the host at natural scale; the fp8 rounding
    noise is diluted by softmax averaging (Z ~ 700) everywhere it matters.
  - The 128 term-token V rows pass straight to the output, so that slice
    (mt=5) is computed in bf16 from a bf16 copy of X's term columns.
  - Score matmuls also run fp8 DoubleRow: Q^T/K^T are stored as
    [32 partitions, 2 dh-planes, tokens] per head (W columns host-permuted
    so each projection out-tile is (4 heads x 32 dh-low | dh-high)), making
    the dh=64 contraction a 2-plane fp8 contraction. Because dual-fp8
    matmuls must write PSUM starting at partition 0, block-diagonal scores
    use a pair-quadrant form: lhsT spans TWO blocks (128 key rows), rhs is
    one block's 64 q columns, so each matmul yields [128, 64] of which one
    64-row half is wanted (the other half is junk that exp processes and
    PV never reads).
  - Per-block value sums come from host-precomputed per-block column sums
    of X (Xsum [H, 10], padded to 16 for dual-fp8 ldweights alignment)
    via one tiny fp8 matmul chain.
  - One head's scores live in a single 3-bank PSUM tile [128, 1280]
    (terms q0:512 at cols 0:512, blocks at 512+c*64, terms q512:640 at
    1152:1280), exp'd by ONE activation instruction.
  - PSUM: one unified tag of [128, 1536] (3-bank) tiles x2 bufs (12KB) +
    the PV accumulator tag x2 (4KB) = exactly 16KB. Projection tiles hold
    both 512-col chunks of an mt so each PSUM->SBUF copy is one
    instruction for the whole mt.
  - Elementwise is balanced: exps+K->ACT, Q/V/vsum/recip/divide->DVE.
  - Outputs are staged into [128, H] SBUF tiles and DMA'd once per
    128-query row-block (6 output DMAs per batch instead of 16).

PSUM discipline: start=True lazily zeroes the whole bank for the written
partitions; every dual-fp8 matmul writes full-height at partition 0, so
each bank's group is opened by its first full-height matmul and closed by
stop=True on its last.
"""

import numpy as np
import ml_dtypes

import concourse.bass as bass
import concourse.mybir as mybir
import concourse.tile as tile
from concourse import bacc
from concourse.bass_utils import run_bass_kernel_spmd

# Problem dims (hardcoded per contract)
B, CDD, L, T, H, NH = 16, 10, 64, 128, 768, 12
DH = H // NH  # 64
S = CDD * L + T  # 768
NQ = CDD * L  # 640
P = 128
NCORES = 8
BL = B // NCORES  # 2 batches per core
KT = H // P  # 6 contraction tiles
KTP = KT // 2  # 3 fp8 DoubleRow contraction pairs
CDDP = 16  # Xsum padded block count (dual-fp8 ldweights alignment)
FP32 = mybir.dt.float32
BF16 = mybir.dt.bfloat16
FP8 = mybir.dt.float8e4
AF = mybir.ActivationFunctionType
ALU = mybir.AluOpType
DR = mybir.MatmulPerfMode.DoubleRow
HGS = 4  # heads per attention group
NHG = NH // HGS  # 3 groups
VW = DH + 1  # value width per head incl. ones column (65)

# score-tile column layout: terms chunk0 | 10 blocks | terms chunk1
SB0 = 512  # blocks start
ST1 = 512 + CDD * L  # terms chunk1 start (1152)
SW = ST1 + (NQ - 512)  # 1280 total


def _tcol(j):
    # column of query chunk j*128 in the score tile's terms region
    return j * P if j < 4 else ST1


def _build_program():
    nc = bacc.Bacc(
        "TRN2", target_bir_lowering=False, debug=False, num_devices=NCORES
    )
    x8 = nc.dram_tensor("x8", [BL, H, S], FP8, kind="ExternalInput").ap()
    xt16 = nc.dram_tensor("xt16", [BL, H, T], BF16, kind="ExternalInput").ap()
    xs8 = nc.dram_tensor("xs8", [BL, H, CDDP], FP8, kind="ExternalInput").ap()
    wq8 = nc.dram_tensor("wq8", [H, H], FP8, kind="ExternalInput").ap()
    wk8 = nc.dram_tensor("wk8", [H, H], FP8, kind="ExternalInput").ap()
    wv8 = nc.dram_tensor("wv8", [H, H], FP8, kind="ExternalInput").ap()
    wv16 = nc.dram_tensor("wv16", [H, H], BF16, kind="ExternalInput").ap()
    bq = nc.dram_tensor("bq", [H], FP32, kind="ExternalInput").ap()
    bk = nc.dram_tensor("bk", [H], FP32, kind="ExternalInput").ap()
    bv16 = nc.dram_tensor("bv16", [H], BF16, kind="ExternalInput").ap()
    out = nc.dram_tensor("out", [BL, S, H], FP32, kind="ExternalOutput").ap()

    with tile.TileContext(nc) as tc:
        _emit(tc, nc, x8, xt16, xs8, wq8, wk8, wv8, wv16, bq, bk, bv16, out)
    nc.compile()
    return nc


def _emit(tc, nc, x8, xt16, xs8, wq8, wk8, wv8, wv16, bq, bk, bv16, out):
    from contextlib import ExitStack

    ctx = ExitStack()
    with ctx:
        cpool = ctx.enter_context(tc.tile_pool(name="consts", bufs=1))
        wpool = ctx.enter_context(tc.tile_pool(name="weights", bufs=1))
        xtp = ctx.enter_context(tc.tile_pool(name="xt", bufs=2))
        qkv = ctx.enter_context(tc.tile_pool(name="qkv", bufs=2))
        sep = ctx.enter_context(tc.tile_pool(name="se", bufs=1))
        osp = ctx.enter_context(tc.tile_pool(name="osb", bufs=2))
        smp = ctx.enter_context(tc.tile_pool(name="small", bufs=2))
        psp = ctx.enter_context(tc.tile_pool(name="psum", bufs=1, space="PSUM"))

        def psbig():
            return psp.tile(
                [P, SW], FP32, tag="psS", bufs=2, name="psS",
                padded_shape=[P, 1536],
            )

        # ---- constants ----
        onesrow = cpool.tile([1, P], BF16)  # 1.0 row (rank-1 lhsT)
        nc.gpsimd.memset(onesrow[:], 1.0)
        zrow = cpool.tile([1, 1], BF16)  # 0.0 (group-closer rank-1 rhs)
        nc.gpsimd.memset(zrow[:], 0.0)
        # notselC[p, c*64+j] = 0 if p == c else 1  (p in 0..9)
        notselC = cpool.tile([CDD, NQ], BF16)
        nc.gpsimd.memset(notselC[:], 1.0)
        nc.gpsimd.affine_select(
            out=notselC.rearrange("p (c j) -> p c j", j=L),
            in_=notselC.rearrange("p (c j) -> p c j", j=L),
            compare_op=ALU.not_equal,
            fill=0.0,
            base=0,
            pattern=[[-1, CDD], [0, L]],
            channel_multiplier=1,
        )

        # ---- weights & biases (shared by both batches) ----
        # x(b=0) first so the first projection can start ASAP, then weights.
        x_cur = {
            "x8": xtp.tile([P, KT, S], FP8, tag="x8", name="x8t"),
            "xt16": xtp.tile([P, KT, T], BF16, tag="xt16", name="xt16t"),
            "xs8": xtp.tile([P, KT, CDDP], FP8, tag="xs8", name="xs8t"),
        }
        nc.sync.dma_start(
            out=x_cur["x8"][:], in_=x8[0].rearrange("(kt p) s -> p kt s", p=P)
        )
        w_sb = {}
        for name, ap_ in (("q", wq8), ("k", wk8), ("v", wv8)):
            t = wpool.tile([P, KT, H], FP8, tag=f"w{name}8", name=f"w{name}8")
            nc.sync.dma_start(
                out=t[:], in_=ap_.rearrange("(kt p) o -> p kt o", p=P)
            )
            w_sb[name] = t
        wv16_sb = wpool.tile([P, KT, H], BF16, tag="wv16", name="wv16sb")
        nc.sync.dma_start(
            out=wv16_sb[:], in_=wv16.rearrange("(kt p) o -> p kt o", p=P)
        )
        nc.sync.dma_start(
            out=x_cur["xt16"][:], in_=xt16[0].rearrange("(kt p) s -> p kt s", p=P)
        )
        nc.sync.dma_start(
            out=x_cur["xs8"][:], in_=xs8[0].rearrange("(kt p) s -> p kt s", p=P)
        )
        b_col = {}
        for name, bap in (("q", bq), ("k", bk)):
            bc = cpool.tile([P, KT], FP32, tag=f"bc{name}", name=f"bcol{name}")
            nc.sync.dma_start(out=bc[:], in_=bap.rearrange("(t p) -> p t", p=P))
            b_col[name] = bc
        bv_row = cpool.tile([1, H], BF16)
        nc.sync.dma_start(out=bv_row[:], in_=bv16[None, :])
        bvb = cpool.tile([P, H], FP32)  # materialized [128, H] V bias

        x_next = x_cur
        for b in range(BL):
            xc = x_next

            # ---- Q/K projections -> fp8 [32, 2(dh-half), tokens] ----
            # W columns are host-permuted: out-tile t=2m+half holds
            # (head 4m + r//32, dh = 32*half + r%32) at partition r.
            q8 = [qkv.tile([P, 2, NQ], FP8, tag=f"q8{m}", name=f"q8{m}") for m in range(3)]
            k8 = [qkv.tile([P, 2, S], FP8, tag=f"k8{m}", name=f"k8{m}") for m in range(3)]
            for name, dst, ntot in (("q", q8, NQ), ("k", k8, S)):
                for t in range(KT):
                    m, half = divmod(t, 2)
                    ts_ = slice(t * P, (t + 1) * P)
                    ps = psbig()
                    for n0 in range(0, ntot, 512):
                        nlen = min(512, ntot - n0)
                        for i in range(KTP):
                            nc.tensor.matmul(
                                ps[:, n0 : n0 + nlen],
                                lhsT=w_sb[name][:, 2 * i : 2 * i + 2, ts_],
                                rhs=xc["x8"][:, 2 * i : 2 * i + 2, n0 : n0 + nlen],
                                start=(i == 0),
                                stop=(i == KTP - 1),
                                perf_mode=DR,
                            )
                    if name == "k":
                        nc.scalar.activation(
                            dst[m][:, half, :],
                            ps[:, 0:ntot],
                            AF.Identity,
                            bias=b_col[name][:, t : t + 1],
                        )
                    else:
                        nc.vector.tensor_scalar_add(
                            dst[m][:, half, :],
                            ps[:, 0:ntot],
                            b_col[name][:, t : t + 1],
                        )

            # ---- scores + exp for one head: single [128, 1280] 3-bank PSUM
            # tile; terms q0:512 at 0:512, blocks at 512+c*64 (pair-quadrant:
            # half the rows junk), terms q512:640 at 1152:1280; ONE exp. ----
            def emit_scores(h):
                m, hl = divmod(h, HGS)
                rows = slice(32 * hl, 32 * hl + 32)
                tp = (32 * hl, 0)
                pss = psbig()
                nc.tensor.matmul(
                    pss[:, 0:512],
                    lhsT=k8[m][rows, :, NQ:S],
                    rhs=q8[m][rows, :, 0:512],
                    start=True,
                    stop=True,
                    perf_mode=DR,
                    tile_position=tp,
                )
                for c in range(CDD):
                    j = c // 2
                    js = slice(2 * j * L, (2 * j + 2) * L)  # the block PAIR's keys
                    cs = slice(c * L, (c + 1) * L)
                    nc.tensor.matmul(
                        pss[:, SB0 + c * L : SB0 + (c + 1) * L],
                        lhsT=k8[m][rows, :, js],
                        rhs=q8[m][rows, :, cs],
                        start=(c in (0, 8)),  # cols 512 and 1024 open banks 1, 2
                        stop=(c == 7),  # last matmul touching bank 1
                        perf_mode=DR,
                        skip_group_check=True,
                        tile_position=tp,
                    )
                nc.tensor.matmul(
                    pss[:, ST1:SW],
                    lhsT=k8[m][rows, :, NQ:S],
                    rhs=q8[m][rows, :, 512:640],
                    start=False,
                    stop=True,
                    perf_mode=DR,
                    skip_group_check=True,
                    tile_position=tp,
                )
                se = sep.tile([P, SW], BF16, tag=f"se{h}", name=f"se{h}")
                nc.scalar.activation(se[:], pss[:, 0:SW], AF.Exp, scale=0.125)
                return se

            se_h = [None] * NH
            se_h[0] = emit_scores(0)

            # ---- V projection (interleaved with first head group's scores
            # so exp latency hides behind PE work) ----
            vext = [qkv.tile([P, NH * VW], BF16, tag=f"v{mt}", name=f"v{mt}") for mt in range(KT)]
            vterm = qkv.tile([P, H], FP32, tag="vterm", name="vterm")
            if b == 0:
                # materialized V bias [128, H] fp32 (free-dim bias add on copy)
                ps = psbig()
                for n0, nlen in ((0, 512), (512, 256)):
                    nc.tensor.matmul(
                        ps[:, n0 : n0 + nlen],
                        lhsT=onesrow[:],
                        rhs=bv_row[0:1, n0 : n0 + nlen],
                        start=True,
                        stop=True,
                    )
                nc.vector.tensor_copy(bvb[:], ps[:, 0:H])

            def v_copy(mt, ps):
                vv = vext[mt].rearrange("p (h c) -> p h c", c=VW)
                nc.vector.tensor_tensor(
                    out=vv[:, :, 0:DH],
                    in0=ps[:, 0:H].rearrange("p (h c) -> p h c", c=DH),
                    in1=bvb[:].rearrange("p (h c) -> p h c", c=DH),
                    op=ALU.add,
                )

            # mt=5: bf16 (term rows feed the output directly)
            ps = psbig()
            for n0, nlen in ((0, 512), (512, 256)):
                for kt in range(KT):
                    nc.tensor.matmul(
                        ps[:, n0 : n0 + nlen],
                        lhsT=xc["xt16"][:, kt, :],
                        rhs=wv16_sb[:, kt, n0 : n0 + nlen],
                        start=(kt == 0),
                        stop=(kt == KT - 1),
                    )
            v_copy(5, ps)
            nc.vector.tensor_tensor(
                out=vterm[:], in0=ps[:, 0:H], in1=bvb[:], op=ALU.add
            )
            # term rows pass through V - DMA out early
            nc.sync.dma_start(out=out[b][NQ:S, :], in_=vterm[:])

            se_h[1] = emit_scores(1)

            # candidate rows mt=0..4: fp8 DoubleRow
            for mt in range(5):
                ms = slice(mt * P, (mt + 1) * P)
                ps = psbig()
                for n0, nlen in ((0, 512), (512, 256)):
                    for i in range(KTP):
                        nc.tensor.matmul(
                            ps[:, n0 : n0 + nlen],
                            lhsT=xc["x8"][:, 2 * i : 2 * i + 2, ms],
                            rhs=w_sb["v"][:, 2 * i : 2 * i + 2, n0 : n0 + nlen],
                            start=(i == 0),
                            stop=(i == KTP - 1),
                            perf_mode=DR,
                        )
                v_copy(mt, ps)
                if mt == 0:
                    se_h[2] = emit_scores(2)
                elif mt == 2:
                    se_h[3] = emit_scores(3)
            for mt in range(KT):
                vv = vext[mt].rearrange("p (h c) -> p h c", c=VW)
                nc.gpsimd.memset(vv[:, :, DH : DH + 1], 1.0)

            # ---- per-block value sums from host-precomputed Xsum:
            # Vsum_c = Xsum_c @ Wv + 64*bv; 65th col = 64.0 so the notselC
            # correction matmul also contributes 9*64 to Z. ----
            vsumsE = smp.tile([CDD, NH * VW], BF16, tag="vsums", name="vsumsE")
            ps = psbig()
            for n0, nlen in ((0, 512), (512, 256)):
                for i in range(KTP):
                    nc.tensor.matmul(
                        ps[0:CDD, n0 : n0 + nlen],
                        lhsT=xc["xs8"][:, 2 * i : 2 * i + 2, 0:CDD],
                        rhs=w_sb["v"][:, 2 * i : 2 * i + 2, n0 : n0 + nlen],
                        start=(i == 0),
                        stop=(i == KTP - 1),
                        perf_mode=DR,
                    )
            vsv = vsumsE.rearrange("p (h c) -> p h c", c=VW)
            nc.vector.scalar_tensor_tensor(
                out=vsv[:, :, 0:DH],
                in0=bvb[0:CDD, :].rearrange("p (h c) -> p h c", c=DH),
                scalar=float(L),
                in1=ps[0:CDD, 0:H].rearrange("p (h c) -> p h c", c=DH),
                op0=ALU.mult,
                op1=ALU.add,
            )
            nc.gpsimd.memset(vsv[:, :, DH : DH + 1], float(L))

            # prefetch next batch's inputs while attention runs
            if b + 1 < BL:
                x_next = {
                    "x8": xtp.tile([P, KT, S], FP8, tag="x8", name="x8t"),
                    "xt16": xtp.tile([P, KT, T], BF16, tag="xt16", name="xt16t"),
                    "xs8": xtp.tile([P, KT, CDDP], FP8, tag="xs8", name="xs8t"),
                }
                nc.sync.dma_start(
                    out=x_next["x8"][:],
                    in_=x8[b + 1].rearrange("(kt p) s -> p kt s", p=P),
                )
                nc.sync.dma_start(
                    out=x_next["xt16"][:],
                    in_=xt16[b + 1].rearrange("(kt p) s -> p kt s", p=P),
                )
                nc.sync.dma_start(
                    out=x_next["xs8"][:],
                    in_=xs8[b + 1].rearrange("(kt p) s -> p kt s", p=P),
                )

            # ---- attention PV + output staging ----
            ostage = [
                osp.tile([P, H], FP32, tag=f"os{j}", name=f"os{j}")
                for j in range(5)
            ]

            def emit_pv_j(hg, j):
                hgs_v = slice(hg * HGS * VW, (hg + 1) * HGS * VW)
                psc = psp.tile(
                    [P, HGS * VW], FP32, tag="psC", bufs=2, name="psC",
                    padded_shape=[P, 512],
                )
                # head 0's full-height terms matmul opens the bank's one
                # accumulation group; everything else accumulates.
                for hl in range(HGS):
                    h = hg * HGS + hl
                    vs = slice(h * VW, (h + 1) * VW)
                    nc.tensor.matmul(
                        psc[:, hl * VW : (hl + 1) * VW],
                        lhsT=se_h[h][:, _tcol(j) : _tcol(j) + P],
                        rhs=vext[5][:, vs],
                        start=(hl == 0),
                        stop=False,
                    )
                for half in (0, 1):
                    c = 2 * j + half
                    hs = slice(half * 64, half * 64 + 64)
                    nc.tensor.matmul(
                        psc[hs, :],
                        lhsT=notselC[:, c * L : (c + 1) * L],
                        rhs=vsumsE[:, hgs_v],
                        start=False,
                        stop=False,
                    )
                for hl in range(HGS):
                    h = hg * HGS + hl
                    vs = slice(h * VW, (h + 1) * VW)
                    for half in (0, 1):
                        c = 2 * j + half
                        hs = slice(half * 64, half * 64 + 64)
                        nc.tensor.matmul(
                            psc[hs, hl * VW : hl * VW + VW],
                            lhsT=se_h[h][hs, SB0 + c * L : SB0 + (c + 1) * L],
                            rhs=vext[j][hs, vs],
                            start=False,
                            stop=False,
                        )
                # full-height +0 rank-1 whose stop closes the bank's group
                nc.tensor.matmul(
                    psc[:, DH : DH + 1],
                    lhsT=onesrow[:],
                    rhs=zrow[:],
                    start=False,
                    stop=True,
                )
                zr = smp.tile([P, HGS], FP32, tag="zr", bufs=4, name="zr")
                pscv = psc.rearrange("p (h c) -> p h c", c=VW)
                nc.vector.reciprocal(
                    zr[:].rearrange("p (h o) -> p h o", o=1),
                    pscv[:, :, DH : DH + 1],
                )
                ov = ostage[j].rearrange("p (h c) -> p h c", c=DH)
                in0 = pscv[:, :, 0:DH]
                in1 = zr[:].rearrange("p (h o) -> p h o", o=1)
                bin0, bin1 = bass.broadcast_tensor_aps(in0, in1)
                nc.vector.tensor_tensor(
                    out=ov[:, hg * HGS : (hg + 1) * HGS, :],
                    in0=bin0,
                    in1=bin1,
                    op=ALU.mult,
                )
                if hg == NHG - 1:
                    nc.sync.dma_start(
                        out=out[b][j * P : (j + 1) * P, :],
                        in_=ostage[j][:],
                    )

            # interleave remaining scores with PV chunks so exp latency and
            # the 2-slot PSUM rotation stay off the PE critical path
            for hg in range(1, NHG):
                for j in range(5):
                    h = hg * HGS + j
                    if j < HGS:
                        se_h[h] = emit_scores(h)
                    emit_pv_j(hg - 1, j)
            emit_pv = lambda j: emit_pv_j(NHG - 1, j)
            for j in range(5):
                emit_pv(j)


_CACHE = {}


def _get_program():
    if "nc" not in _CACHE:
        _CACHE["nc"] = _build_program()
    return _CACHE["nc"]


NPF8 = ml_dtypes.float8_e4m3
NPBF = ml_dtypes.bfloat16


def _make_in_maps(inputs):
    hs = np.asarray(inputs["hidden_states"], np.float32)
    hst = np.ascontiguousarray(hs.transpose(0, 2, 1))  # [B, H, S]
    x8 = hst.astype(NPF8)
    xt16 = np.ascontiguousarray(hst[:, :, NQ:]).astype(NPBF)
    xsum = np.zeros((B, H, CDDP), np.float32)
    xsum[:, :, :CDD] = hst[:, :, :NQ].reshape(B, H, CDD, L).sum(axis=3)
    xs8 = xsum.astype(NPF8)

    # Q/K projection out-column permutation: out-tile t=2m+half holds
    # (head 4m + r//32, dh = 32*half + r%32) at partition r.
    r = np.arange(P)
    perm = np.empty(H, np.int64)
    for t in range(KT):
        m, half = divmod(t, 2)
        perm[t * P + r] = (HGS * m + r // 32) * DH + 32 * half + (r % 32)

    wq = np.asarray(inputs["Wq"], np.float32).T
    wk = np.asarray(inputs["Wk"], np.float32).T
    wv = np.asarray(inputs["Wv"], np.float32).T
    bqp = np.asarray(inputs["bq"], np.float32)[perm]
    bkp = np.asarray(inputs["bk"], np.float32)[perm]
    bv = np.asarray(inputs["bv"], np.float32)
    in_common = {
        "wq8": np.ascontiguousarray(wq[:, perm]).astype(NPF8),
        "wk8": np.ascontiguousarray(wk[:, perm]).astype(NPF8),
        "wv8": np.ascontiguousarray(wv).astype(NPF8),
        "wv16": np.ascontiguousarray(wv).astype(NPBF),
        "bq": np.ascontiguousarray(bqp),
        "bk": np.ascontiguousarray(bkp),
        "bv16": bv.astype(NPBF),
    }
    return [
        {
            "x8": x8[i * BL : (i + 1) * BL],
            "xt16": xt16[i * BL : (i + 1) * BL],
            "xs8": xs8[i * BL : (i + 1) * BL],
            **in_common,
        }
        for i in range(NCORES)
    ]


def kernel(**inputs) -> np.ndarray:
    in_maps = _make_in_maps(inputs)
    nc = _get_program()
    res = run_bass_kernel_spmd(nc, in_maps, list(range(NCORES)))
    return np.concatenate([res.results[i]["out"] for i in range(NCORES)], axis=0)
